# revision 17
# baseline (speedup 1.0000x reference)
"""MoE routed dynamics kernel for Trainium2 (8 NeuronCores, expert-parallel).

Problem: for each row b of a [B, D+A] input, route through one of P=8
two-layer MLPs selected by policy_indices[b]:
    h = relu(x @ W1[p] + b1[p]);  y = h @ W2[p] + b2[p]

Sharding: expert-parallel. Core p owns expert p's weights (resident in
SBUF) and processes exactly the rows routed to expert p. The all-to-all
dispatch keyed on policy_indices happens on the host at shard time
(gather rows by expert, pad to a common capacity C), and the inverse
scatter happens at unshard time.

Device kernel (per core), all activations feature-on-partition so no
transposes are needed anywhere:
    xT   [DA, C]  (DA=576)         input, transposed on host
    hT   [H, C]   = relu(W1.T @ x + b1), H=1024, via PE matmuls
    outT [D, C]   = W2.T @ h + b2,  D=512
Matmuls run as out[M,N] = lhsT.T @ rhs with lhsT = weight chunks in
their natural [K, M] layout and rhs = activation chunks [K, N<=512].

fp32r matmuls stream at 1 col/cycle (same as bf16) when the moving dim
is >= 256, so everything stays full-fp32-precision float32r and all
column chunks are kept >= 256 wide.

Layer-1 contraction is K = 576 = 4*128 + 64.  The ragged 64-row tail
is handled by row-packing: the 64 tail rows of x are duplicated into
partitions 64:128 (done on the host), and the tail matmuls for two
adjacent output tiles (m=2j, m=2j+1) run concurrently on row-groups
0:64 / 64:128 of the PE array via tile_position -- one N-cycle span
instead of two per pair.

DMA issue occupies the issuing engine's queue ~0.6us per dma_start
(FIFO per engine, transfers async), so the plan is: chunk-0 x tiles
interleaved with the first-needed half of W1 on Sync (fine-grained, so
the first matmul starts ~1.5us after the framework preamble), all
remaining weights as 4 big merged DMAs on Scalar, remaining x chunks
on Sync, output stores on Scalar.
"""

import math
import os

import numpy as np

_B = 16384
_P = 8
_D = 512
_A = 64
_H = 1024
_DA = _D + _A   # 576
_KF = 4         # full 128-row K chunks of layer 1
_N_CORES = 8

_MM_DTYPE = os.environ.get("MM_DTYPE", "bfloat16")
# row-packed layer-1 tail matmuls via tile_position (env-toggleable for debug)
_PAIR_TAIL = os.environ.get("PAIR_TAIL", "1") == "1"

_kernel_cache: dict = {}


def _n_chunks(C: int):
    """Column chunking: all chunks >= 256 (fp32r full-rate), <= 512 (one
    PSUM bank). Smallest chunk last (short kernel tail), second-smallest
    first (fast DMA-paced warm-up)."""
    sizes = []
    rem = C
    while rem > 1023:
        sizes.append(512)
        rem -= 512
    if rem >= 768:
        sizes.extend([512, rem - 512])
    else:
        sizes.extend([rem - 256, 256])
    order = sorted(sizes, reverse=True)   # big chunks first, smallest last
    out = []
    n0 = 0
    for nl in order:
        out.append((n0, nl))
        n0 += nl
    assert n0 == C and all(256 <= nl <= 512 for _, nl in out), (C, out)
    return out


def _build_bass(C: int):
    import concourse.bacc as bacc
    import concourse.mybir as mybir
    from concourse.tile import TileContext

    fp32 = mybir.dt.float32
    mmdt = getattr(mybir.dt, _MM_DTYPE)
    act = mybir.ActivationFunctionType

    n_chunks = _n_chunks(C)
    mh = _H // 128        # 8 output tiles of layer 1
    md = _D // 128        # 4 output tiles of layer 2
    kh = _H // 128        # 8 K chunks of layer 2

    nc = bacc.Bacc()
    # x: 4 full K chunks + the 64-row tail duplicated into both halves
    xT = nc.declare_dram_parameter("xT", [_KF + 1, 128, C], mmdt, isOutput=False)
    # W1 full chunks [4, 128, H]; tail pairs [128, 4*128] (rows 0:64 =
    # W1[512:576] cols of tile 2j, rows 64:128 = cols of tile 2j+1)
    w1f = nc.declare_dram_parameter("w1f", [_KF, 128, _H], mmdt, isOutput=False)
    w1t = nc.declare_dram_parameter("w1t", [128, (mh // 2) * 128], mmdt, isOutput=False)
    w2 = nc.declare_dram_parameter("w2", [kh, 128, _D], mmdt, isOutput=False)
    # biases packed together: cols 0:mh = b1 tiles, mh:mh+md = b2 tiles
    b12 = nc.declare_dram_parameter("b12", [128, mh + md], fp32, isOutput=False)
    # Output stays in the matmul dtype (bf16 halves store traffic; host
    # upcasts). fp32 PSUM -> bf16 rounding adds ~2e-4 relative error.
    outT = nc.declare_dram_parameter("outT", [md, 128, C], mmdt, isOutput=True)

    with TileContext(nc) as tc:
        with (
            tc.tile_pool(name="wpool", bufs=1) as wpool,
            tc.tile_pool(name="xpool", bufs=3) as xpool,
            tc.tile_pool(name="hpool", bufs=2) as hpool,
            tc.tile_pool(name="ypool", bufs=3) as ypool,
            tc.tile_pool(name="ps1", bufs=4, space="PSUM") as ps1,
            tc.tile_pool(name="ps2", bufs=4, space="PSUM") as ps2,
        ):
            w1_tiles = [
                wpool.tile([128, _H], mmdt, name=f"w1_{k}", tag=f"w1_{k}")
                for k in range(_KF)
            ]

            def w1s(k, m):
                return w1_tiles[k][:, m * 128 : (m + 1) * 128]

            w2_tiles = [
                wpool.tile([128, _D], mmdt, name=f"w2_{k}", tag=f"w2_{k}")
                for k in range(kh)
            ]

            def w2s(k, d):
                return w2_tiles[k][:, d * 128 : (d + 1) * 128]

            # --- DMA issue plan ------------------------------------------
            # All loads share the Sync HWDGE ring so HBM delivery follows
            # issue order exactly (a second load ring would round-robin
            # packets and delay the critical head tiles). Order = order of
            # first compute use: chunk-0 x interleaved with W1 cols 0:512
            # (feeds L1 pairs j=0,1), W1 cols 512:1024, x chunk 1, W2
            # (needed only when L2(c0) runs, after L1(c1)), then x2..x4.
            n0_0, nl_0 = n_chunks[0]
            x_first = []
            for k in range(_KF):
                xt = xpool.tile([128, nl_0], mmdt, tag=f"x_{k}")
                nc.sync.dma_start(out=xt[:, :], in_=xT[k, :, n0_0 : n0_0 + nl_0])
                x_first.append(xt)
                nc.sync.dma_start(
                    out=w1_tiles[k][:, : _H // 2], in_=w1f[k, :, : _H // 2]
                )
            xt = xpool.tile([128, nl_0], mmdt, tag=f"x_{_KF}")
            nc.sync.dma_start(out=xt[:, :], in_=xT[_KF, :, n0_0 : n0_0 + nl_0])
            x_first.append(xt)

            # Scalar: just the two small early-needed tiles (biases + W1
            # tail pairs); its queue must stay free for the relus.
            b12_sb = wpool.tile([128, mh + md], fp32, tag="b12")
            nc.scalar.dma_start(out=b12_sb[:], in_=b12[:, :])
            w1t_sb = wpool.tile([128, (mh // 2) * 128], mmdt, tag="w1t")
            nc.scalar.dma_start(out=w1t_sb[:], in_=w1t[:, :])

            def dma_x(n0, nl):
                tiles = []
                for k in range(_KF + 1):
                    t = xpool.tile([128, nl], mmdt, tag=f"x_{k}")
                    nc.sync.dma_start(out=t[:, :], in_=xT[k, :, n0 : n0 + nl])
                    tiles.append(t)
                return tiles

            for k in range(_KF):
                nc.sync.dma_start(
                    out=w1_tiles[k][:, _H // 2 :], in_=w1f[k, :, _H // 2 :]
                )
            x_all = [x_first, dma_x(*n_chunks[1])]
            for k in range(kh):
                nc.sync.dma_start(out=w2_tiles[k][:, :], in_=w2[k, :, :])
            x_all += [dma_x(n0, nl) for n0, nl in n_chunks[2:]]

            # --- PE warm-up ----------------------------------------------
            # The PE HAM clock gate only reaches 8/8 (2.4 GHz) after ~3.4us
            # of sustained activity. Real matmuls can't start until the
            # first x/w tiles land (~4us after the preamble), so burn that
            # DMA-wait on junk matmuls over a memset scratch tile: by the
            # time data arrives the PE is already at full clock.
            warm = wpool.tile([128, 256], mmdt, tag="warm")
            nc.vector.memset(warm[:, :], 0)
            # Scratch PSUM from the ps2 pool (first real ps2 use is ~15us
            # later, so the WAW dep on the warm-up group never stalls).
            # 23 x N=256 at the cold 1.2 GHz clock ~= 5us of PE activity,
            # which covers the gap until the first x/w tiles land (~12.5us:
            # ~7us preamble+issue plus ~4-5us DMA completion latency while
            # all 8 cores hammer HBM at once).
            wps = ps2.tile([128, 256], fp32, tag="ps2")
            for i in range(23):
                nc.tensor.matmul(
                    wps[:, :], warm[:, 0:128], warm[:, :],
                    start=(i == 0), stop=(i == 22),
                )

            # --- compute -------------------------------------------------
            def l1(ci):
                n0, nl = n_chunks[ci]
                x_sb = x_all[ci]
                h_sb = [None] * mh
                for j in range(mh // 2):
                    ma, mb = 2 * j, 2 * j + 1
                    psa = ps1.tile([128, nl], fp32, tag="ps1")
                    for k in range(_KF):
                        nc.tensor.matmul(
                            psa[:, :], w1s(k, ma), x_sb[k][:, :],
                            start=(k == 0), stop=False,
                        )
                    psb = ps1.tile([128, nl], fp32, tag="ps1")
                    for k in range(_KF):
                        nc.tensor.matmul(
                            psb[:, :], w1s(k, mb), x_sb[k][:, :],
                            start=(k == 0), stop=False,
                        )
                    # Ragged K=64 tails for tiles (2j, 2j+1): adjacent in
                    # the queue, on disjoint PE row-groups so they overlap.
                    tp_a = (0, 0) if _PAIR_TAIL else None
                    tp_b = (64, 0) if _PAIR_TAIL else None
                    nc.tensor.matmul(
                        psa[:, :],
                        w1t_sb[0:64, j * 128 : (j + 1) * 128],
                        x_sb[_KF][0:64, :],
                        start=False, stop=True, tile_position=tp_a,
                    )
                    nc.tensor.matmul(
                        psb[:, :],
                        w1t_sb[64:128, j * 128 : (j + 1) * 128],
                        x_sb[_KF][64:128, :],
                        start=False, stop=True, tile_position=tp_b,
                    )
                    for m, ps in ((ma, psa), (mb, psb)):
                        ht = hpool.tile([128, nl], mmdt, tag=f"h_{m}")
                        nc.scalar.activation(
                            ht[:], ps[:], act.Relu, bias=b12_sb[:, m : m + 1]
                        )
                        h_sb[m] = ht
                return h_sb

            def l2(ci, h_sb):
                n0, nl = n_chunks[ci]
                last = ci == len(n_chunks) - 1
                for d in range(md):
                    ps = ps2.tile([128, nl], fp32, tag="ps2")
                    for m in range(mh):
                        nc.tensor.matmul(
                            ps[:, :], w2s(m, d), h_sb[m][:, :],
                            start=(m == 0), stop=(m == mh - 1),
                        )
                    yt = ypool.tile([128, nl], mmdt, tag="y")
                    # Bias-add: DVE normally; for the final chunk alternate
                    # DVE/ACT so the last few adds don't serialize on one
                    # engine right at the kernel tail.
                    if last and d % 2 == 1:
                        nc.scalar.activation(
                            yt[:, :], ps[:, :], act.Identity,
                            bias=b12_sb[:, mh + d : mh + d + 1],
                        )
                    else:
                        nc.vector.tensor_scalar_add(
                            yt[:, :], ps[:, :], b12_sb[:, mh + d : mh + d + 1]
                        )
                    # Stores issue from Scalar (Sync's ring is busy with
                    # loads) -- except the final chunk, where Sync is idle
                    # and Scalar would serialize the last stores.
                    seng = nc.sync if last else nc.scalar
                    seng.dma_start(out=outT[d, :, n0 : n0 + nl], in_=yt[:, :])

            # Software-pipelined by one chunk: L1 of chunk c+1 runs before
            # L2 of chunk c, so W2's arrival (2 MB after W1+x0+x1) and each
            # x chunk hide behind compute.
            nch = len(n_chunks)
            h_prev = l1(0)
            for ci in range(1, nch):
                h_cur = l1(ci)
                l2(ci - 1, h_prev)
                h_prev = h_cur
            l2(nch - 1, h_prev)

    nc.compile()
    return nc


def _get_bass(C: int):
    nc = _kernel_cache.get(C)
    if nc is None:
        nc = _build_bass(C)
        _kernel_cache[C] = nc
    return nc


def _mm_np(a):
    """Cast a float32 array to the numpy dtype matching _MM_DTYPE."""
    if _MM_DTYPE == "bfloat16":
        import ml_dtypes

        return np.ascontiguousarray(a.astype(ml_dtypes.bfloat16))
    return np.ascontiguousarray(a)


def _prepare_in_maps(latents, actions, policy_indices, W1, b1, W2, b2):
    """Expert-parallel dispatch: returns (in_maps, C, order, offs, counts)."""
    latents = np.asarray(latents, dtype=np.float32)
    actions = np.asarray(actions, dtype=np.float32)
    pi = np.asarray(policy_indices).astype(np.int64)
    W1 = np.asarray(W1, dtype=np.float32)
    b1 = np.asarray(b1, dtype=np.float32)
    W2 = np.asarray(W2, dtype=np.float32)
    b2 = np.asarray(b2, dtype=np.float32)

    B = latents.shape[0]
    counts = np.bincount(pi, minlength=_P)
    order = np.argsort(pi, kind="stable")
    offs = np.concatenate(([0], np.cumsum(counts)))

    # Per-core capacity: smallest multiple of 64 >= max rows per expert
    # (>= 1536 so the chunking always yields >=256-wide chunks).
    C = max(1536, int(math.ceil(counts.max() / 64)) * 64)

    x = np.empty((B, _DA), dtype=np.float32)
    x[:, :_D] = latents
    x[:, _D:] = actions
    x_sorted = x[order]

    mh = _H // 128
    md = _D // 128
    in_maps = []
    for p in range(_P):
        cp = counts[p]
        xp = np.zeros((_KF + 1, 128, C), dtype=np.float32)
        xs = x_sorted[offs[p] : offs[p + 1]].T          # [576, cp]
        xp[:_KF, :, :cp] = xs[: 4 * 128].reshape(_KF, 128, cp)
        xp[_KF, 0:64, :cp] = xs[4 * 128 :]
        xp[_KF, 64:128, :cp] = xs[4 * 128 :]            # duplicated tail
        w1p = W1[p]                                     # [576, 1024]
        w1fp = w1p[: 4 * 128].reshape(_KF, 128, _H)
        w1tp = np.zeros((128, (mh // 2) * 128), dtype=np.float32)
        tail = w1p[4 * 128 :]                           # [64, 1024]
        for j in range(mh // 2):
            w1tp[0:64, j * 128 : (j + 1) * 128] = tail[:, (2 * j) * 128 : (2 * j + 1) * 128]
            w1tp[64:128, j * 128 : (j + 1) * 128] = tail[:, (2 * j + 1) * 128 : (2 * j + 2) * 128]
        b12p = np.concatenate(
            [b1[p].reshape(mh, 128).T, b2[p].reshape(md, 128).T], axis=1
        )
        in_maps.append(
            {
                "xT": _mm_np(xp),
                "w1f": _mm_np(w1fp),
                "w1t": _mm_np(w1tp),
                "w2": _mm_np(W2[p].reshape(_H // 128, 128, _D)),
                "b12": np.ascontiguousarray(b12p),
            }
        )
    return in_maps, C, order, offs, counts


def kernel(latents, actions, policy_indices, W1, b1, W2, b2):
    from concourse.bass_utils import run_bass_kernel_spmd

    in_maps, C, order, offs, counts = _prepare_in_maps(
        latents, actions, policy_indices, W1, b1, W2, b2
    )
    nc = _get_bass(C)
    results = run_bass_kernel_spmd(nc, in_maps, list(range(_N_CORES))).results

    B = np.asarray(latents).shape[0]
    out = np.empty((B, _D), dtype=np.float32)
    for p in range(_P):
        yT = np.asarray(results[p]["outT"], dtype=np.float32).reshape(_D, C)
        out[order[offs[p] : offs[p + 1]]] = yT[:, : counts[p]].T
    return out


# revision 18
# speedup vs baseline: 1.0062x; 1.0062x over previous
"""MoE routed dynamics kernel for Trainium2 (8 NeuronCores, expert-parallel).

Problem: for each row b of a [B, D+A] input, route through one of P=8
two-layer MLPs selected by policy_indices[b]:
    h = relu(x @ W1[p] + b1[p]);  y = h @ W2[p] + b2[p]

Sharding: expert-parallel. Core p owns expert p's weights (resident in
SBUF) and processes exactly the rows routed to expert p. The all-to-all
dispatch keyed on policy_indices happens on the host at shard time
(gather rows by expert, pad to a common capacity C), and the inverse
scatter happens at unshard time.

Device kernel (per core), all activations feature-on-partition so no
transposes are needed anywhere:
    xT   [DA, C]  (DA=576)         input, transposed on host
    hT   [H, C]   = relu(W1.T @ x + b1), H=1024, via PE matmuls
    outT [D, C]   = W2.T @ h + b2,  D=512
Matmuls run as out[M,N] = lhsT.T @ rhs with lhsT = weight chunks in
their natural [K, M] layout and rhs = activation chunks [K, N<=512].

fp32r matmuls stream at 1 col/cycle (same as bf16) when the moving dim
is >= 256, so everything stays full-fp32-precision float32r and all
column chunks are kept >= 256 wide.

Layer-1 contraction is K = 576 = 4*128 + 64.  The ragged 64-row tail
is handled by row-packing: the 64 tail rows of x are duplicated into
partitions 64:128 (done on the host), and the tail matmuls for two
adjacent output tiles (m=2j, m=2j+1) run concurrently on row-groups
0:64 / 64:128 of the PE array via tile_position -- one N-cycle span
instead of two per pair.

DMA issue occupies the issuing engine's queue ~0.6us per dma_start
(FIFO per engine, transfers async), so the plan is: chunk-0 x tiles
interleaved with the first-needed half of W1 on Sync (fine-grained, so
the first matmul starts ~1.5us after the framework preamble), all
remaining weights as 4 big merged DMAs on Scalar, remaining x chunks
on Sync, output stores on Scalar.
"""

import math
import os

import numpy as np

_B = 16384
_P = 8
_D = 512
_A = 64
_H = 1024
_DA = _D + _A   # 576
_KF = 4         # full 128-row K chunks of layer 1
_N_CORES = 8

_MM_DTYPE = os.environ.get("MM_DTYPE", "bfloat16")
# row-packed layer-1 tail matmuls via tile_position (env-toggleable for debug)
_PAIR_TAIL = os.environ.get("PAIR_TAIL", "1") == "1"

_kernel_cache: dict = {}


def _n_chunks(C: int):
    """Column chunking: all chunks >= 256 (fp32r full-rate), <= 512 (one
    PSUM bank). Smallest chunk last (short kernel tail), second-smallest
    first (fast DMA-paced warm-up)."""
    sizes = []
    rem = C
    while rem > 1023:
        sizes.append(512)
        rem -= 512
    if rem >= 768:
        sizes.extend([512, rem - 512])
    else:
        sizes.extend([rem - 256, 256])
    order = sorted(sizes, reverse=True)   # big chunks first, smallest last
    out = []
    n0 = 0
    for nl in order:
        out.append((n0, nl))
        n0 += nl
    assert n0 == C and all(256 <= nl <= 512 for _, nl in out), (C, out)
    return out


def _build_bass(C: int):
    import concourse.bacc as bacc
    import concourse.mybir as mybir
    from concourse.tile import TileContext

    fp32 = mybir.dt.float32
    mmdt = getattr(mybir.dt, _MM_DTYPE)
    act = mybir.ActivationFunctionType

    n_chunks = _n_chunks(C)
    mh = _H // 128        # 8 output tiles of layer 1
    md = _D // 128        # 4 output tiles of layer 2
    kh = _H // 128        # 8 K chunks of layer 2

    nc = bacc.Bacc()
    # x: 4 full K chunks + the 64-row tail duplicated into both halves
    xT = nc.declare_dram_parameter("xT", [_KF + 1, 128, C], mmdt, isOutput=False)
    # W1 full chunks [4, 128, H]; tail pairs [128, 4*128] (rows 0:64 =
    # W1[512:576] cols of tile 2j, rows 64:128 = cols of tile 2j+1)
    w1f = nc.declare_dram_parameter("w1f", [_KF, 128, _H], mmdt, isOutput=False)
    w1t = nc.declare_dram_parameter("w1t", [128, (mh // 2) * 128], mmdt, isOutput=False)
    w2 = nc.declare_dram_parameter("w2", [kh, 128, _D], mmdt, isOutput=False)
    # biases packed together: cols 0:mh = b1 tiles, mh:mh+md = b2 tiles
    b12 = nc.declare_dram_parameter("b12", [128, mh + md], fp32, isOutput=False)
    # Output stays in the matmul dtype (bf16 halves store traffic; host
    # upcasts). fp32 PSUM -> bf16 rounding adds ~2e-4 relative error.
    outT = nc.declare_dram_parameter("outT", [md, 128, C], mmdt, isOutput=True)

    with TileContext(nc) as tc:
        with (
            tc.tile_pool(name="wpool", bufs=1) as wpool,
            tc.tile_pool(name="xpool", bufs=3) as xpool,
            tc.tile_pool(name="hpool", bufs=2) as hpool,
            tc.tile_pool(name="ypool", bufs=3) as ypool,
            tc.tile_pool(name="ps1", bufs=5, space="PSUM") as ps1,
            tc.tile_pool(name="ps2", bufs=3, space="PSUM") as ps2,
        ):
            w1_tiles = [
                wpool.tile([128, _H], mmdt, name=f"w1_{k}", tag=f"w1_{k}")
                for k in range(_KF)
            ]

            def w1s(k, m):
                return w1_tiles[k][:, m * 128 : (m + 1) * 128]

            w2_tiles = [
                wpool.tile([128, _D], mmdt, name=f"w2_{k}", tag=f"w2_{k}")
                for k in range(kh)
            ]

            def w2s(k, d):
                return w2_tiles[k][:, d * 128 : (d + 1) * 128]

            # --- DMA issue plan ------------------------------------------
            # All loads share the Sync HWDGE ring so HBM delivery follows
            # issue order exactly (a second load ring would round-robin
            # packets and delay the critical head tiles). Order = order of
            # first compute use: chunk-0 x interleaved with W1 cols 0:512
            # (feeds L1 pairs j=0,1), W1 cols 512:1024, x chunk 1, W2
            # (needed only when L2(c0) runs, after L1(c1)), then x2..x4.
            n0_0, nl_0 = n_chunks[0]
            x_first = []
            for k in range(_KF):
                xt = xpool.tile([128, nl_0], mmdt, tag=f"x_{k}")
                nc.sync.dma_start(out=xt[:, :], in_=xT[k, :, n0_0 : n0_0 + nl_0])
                x_first.append(xt)
                nc.sync.dma_start(
                    out=w1_tiles[k][:, : _H // 2], in_=w1f[k, :, : _H // 2]
                )
            xt = xpool.tile([128, nl_0], mmdt, tag=f"x_{_KF}")
            nc.sync.dma_start(out=xt[:, :], in_=xT[_KF, :, n0_0 : n0_0 + nl_0])
            x_first.append(xt)

            # W1 tail pairs ride Sync too (the Scalar ring pays its own
            # ~4-5us first-DMA ramp at kernel start, which would make w1t
            # late); only the tiny bias tile uses Scalar.
            w1t_sb = wpool.tile([128, (mh // 2) * 128], mmdt, tag="w1t")
            nc.sync.dma_start(out=w1t_sb[:], in_=w1t[:, :])
            b12_sb = wpool.tile([128, mh + md], fp32, tag="b12")
            nc.scalar.dma_start(out=b12_sb[:], in_=b12[:, :])

            def dma_x(n0, nl):
                tiles = []
                for k in range(_KF + 1):
                    t = xpool.tile([128, nl], mmdt, tag=f"x_{k}")
                    nc.sync.dma_start(out=t[:, :], in_=xT[k, :, n0 : n0 + nl])
                    tiles.append(t)
                return tiles

            for k in range(_KF):
                nc.sync.dma_start(
                    out=w1_tiles[k][:, _H // 2 :], in_=w1f[k, :, _H // 2 :]
                )
            x_all = [x_first, dma_x(*n_chunks[1])]
            for k in range(kh):
                nc.sync.dma_start(out=w2_tiles[k][:, :], in_=w2[k, :, :])
            x_all += [dma_x(n0, nl) for n0, nl in n_chunks[2:]]

            # --- PE warm-up ----------------------------------------------
            # The PE HAM clock gate only reaches 8/8 (2.4 GHz) after ~3.4us
            # of sustained activity. Real matmuls can't start until the
            # first x/w tiles land (~4us after the preamble), so burn that
            # DMA-wait on junk matmuls over a memset scratch tile: by the
            # time data arrives the PE is already at full clock.
            warm = wpool.tile([128, 256], mmdt, tag="warm")
            nc.vector.memset(warm[:, :], 0)
            # Scratch PSUM from the ps2 pool (first real ps2 use is ~15us
            # later, so the WAW dep on the warm-up group never stalls).
            # 23 x N=256 at the cold 1.2 GHz clock ~= 5us of PE activity,
            # which covers the gap until the first x/w tiles land (~12.5us:
            # ~7us preamble+issue plus ~4-5us DMA completion latency while
            # all 8 cores hammer HBM at once).
            wps = ps2.tile([128, 256], fp32, tag="ps2")
            for i in range(23):
                nc.tensor.matmul(
                    wps[:, :], warm[:, 0:128], warm[:, :],
                    start=(i == 0), stop=(i == 22),
                )

            # --- compute -------------------------------------------------
            def l1(ci):
                n0, nl = n_chunks[ci]
                x_sb = x_all[ci]
                h_sb = [None] * mh
                for j in range(mh // 2):
                    ma, mb = 2 * j, 2 * j + 1
                    psa = ps1.tile([128, nl], fp32, tag="ps1")
                    for k in range(_KF):
                        nc.tensor.matmul(
                            psa[:, :], w1s(k, ma), x_sb[k][:, :],
                            start=(k == 0), stop=False,
                        )
                    psb = ps1.tile([128, nl], fp32, tag="ps1")
                    for k in range(_KF):
                        nc.tensor.matmul(
                            psb[:, :], w1s(k, mb), x_sb[k][:, :],
                            start=(k == 0), stop=False,
                        )
                    # Ragged K=64 tails for tiles (2j, 2j+1): adjacent in
                    # the queue, on disjoint PE row-groups so they overlap.
                    tp_a = (0, 0) if _PAIR_TAIL else None
                    tp_b = (64, 0) if _PAIR_TAIL else None
                    nc.tensor.matmul(
                        psa[:, :],
                        w1t_sb[0:64, j * 128 : (j + 1) * 128],
                        x_sb[_KF][0:64, :],
                        start=False, stop=True, tile_position=tp_a,
                    )
                    nc.tensor.matmul(
                        psb[:, :],
                        w1t_sb[64:128, j * 128 : (j + 1) * 128],
                        x_sb[_KF][64:128, :],
                        start=False, stop=True, tile_position=tp_b,
                    )
                    for m, ps in ((ma, psa), (mb, psb)):
                        ht = hpool.tile([128, nl], mmdt, tag=f"h_{m}")
                        nc.scalar.activation(
                            ht[:], ps[:], act.Relu, bias=b12_sb[:, m : m + 1]
                        )
                        h_sb[m] = ht
                return h_sb

            def l2(ci, h_sb):
                n0, nl = n_chunks[ci]
                last = ci == len(n_chunks) - 1
                for d in range(md):
                    ps = ps2.tile([128, nl], fp32, tag="ps2")
                    for m in range(mh):
                        nc.tensor.matmul(
                            ps[:, :], w2s(m, d), h_sb[m][:, :],
                            start=(m == 0), stop=(m == mh - 1),
                        )
                    yt = ypool.tile([128, nl], mmdt, tag="y")
                    # Bias-add: DVE normally; for the final chunk alternate
                    # DVE/ACT so the last few adds don't serialize on one
                    # engine right at the kernel tail.
                    if last and d % 2 == 1:
                        nc.scalar.activation(
                            yt[:, :], ps[:, :], act.Identity,
                            bias=b12_sb[:, mh + d : mh + d + 1],
                        )
                    else:
                        nc.vector.tensor_scalar_add(
                            yt[:, :], ps[:, :], b12_sb[:, mh + d : mh + d + 1]
                        )
                    # Stores issue from Scalar (Sync's ring is busy with
                    # loads) -- except the final chunk, where Sync is idle
                    # and Scalar would serialize the last stores.
                    seng = nc.sync if last else nc.scalar
                    seng.dma_start(out=outT[d, :, n0 : n0 + nl], in_=yt[:, :])

            # Software-pipelined by one chunk: L1 of chunk c+1 runs before
            # L2 of chunk c, so W2's arrival (2 MB after W1+x0+x1) and each
            # x chunk hide behind compute.
            nch = len(n_chunks)
            h_prev = l1(0)
            for ci in range(1, nch):
                h_cur = l1(ci)
                l2(ci - 1, h_prev)
                h_prev = h_cur
            l2(nch - 1, h_prev)

    nc.compile()
    return nc


def _get_bass(C: int):
    nc = _kernel_cache.get(C)
    if nc is None:
        nc = _build_bass(C)
        _kernel_cache[C] = nc
    return nc


def _mm_np(a):
    """Cast a float32 array to the numpy dtype matching _MM_DTYPE."""
    if _MM_DTYPE == "bfloat16":
        import ml_dtypes

        return np.ascontiguousarray(a.astype(ml_dtypes.bfloat16))
    return np.ascontiguousarray(a)


def _prepare_in_maps(latents, actions, policy_indices, W1, b1, W2, b2):
    """Expert-parallel dispatch: returns (in_maps, C, order, offs, counts)."""
    latents = np.asarray(latents, dtype=np.float32)
    actions = np.asarray(actions, dtype=np.float32)
    pi = np.asarray(policy_indices).astype(np.int64)
    W1 = np.asarray(W1, dtype=np.float32)
    b1 = np.asarray(b1, dtype=np.float32)
    W2 = np.asarray(W2, dtype=np.float32)
    b2 = np.asarray(b2, dtype=np.float32)

    B = latents.shape[0]
    counts = np.bincount(pi, minlength=_P)
    order = np.argsort(pi, kind="stable")
    offs = np.concatenate(([0], np.cumsum(counts)))

    # Per-core capacity: smallest multiple of 64 >= max rows per expert
    # (>= 1536 so the chunking always yields >=256-wide chunks).
    C = max(1536, int(math.ceil(counts.max() / 64)) * 64)

    x = np.empty((B, _DA), dtype=np.float32)
    x[:, :_D] = latents
    x[:, _D:] = actions
    x_sorted = x[order]

    mh = _H // 128
    md = _D // 128
    in_maps = []
    for p in range(_P):
        cp = counts[p]
        xp = np.zeros((_KF + 1, 128, C), dtype=np.float32)
        xs = x_sorted[offs[p] : offs[p + 1]].T          # [576, cp]
        xp[:_KF, :, :cp] = xs[: 4 * 128].reshape(_KF, 128, cp)
        xp[_KF, 0:64, :cp] = xs[4 * 128 :]
        xp[_KF, 64:128, :cp] = xs[4 * 128 :]            # duplicated tail
        w1p = W1[p]                                     # [576, 1024]
        w1fp = w1p[: 4 * 128].reshape(_KF, 128, _H)
        w1tp = np.zeros((128, (mh // 2) * 128), dtype=np.float32)
        tail = w1p[4 * 128 :]                           # [64, 1024]
        for j in range(mh // 2):
            w1tp[0:64, j * 128 : (j + 1) * 128] = tail[:, (2 * j) * 128 : (2 * j + 1) * 128]
            w1tp[64:128, j * 128 : (j + 1) * 128] = tail[:, (2 * j + 1) * 128 : (2 * j + 2) * 128]
        b12p = np.concatenate(
            [b1[p].reshape(mh, 128).T, b2[p].reshape(md, 128).T], axis=1
        )
        in_maps.append(
            {
                "xT": _mm_np(xp),
                "w1f": _mm_np(w1fp),
                "w1t": _mm_np(w1tp),
                "w2": _mm_np(W2[p].reshape(_H // 128, 128, _D)),
                "b12": np.ascontiguousarray(b12p),
            }
        )
    return in_maps, C, order, offs, counts


def kernel(latents, actions, policy_indices, W1, b1, W2, b2):
    from concourse.bass_utils import run_bass_kernel_spmd

    in_maps, C, order, offs, counts = _prepare_in_maps(
        latents, actions, policy_indices, W1, b1, W2, b2
    )
    nc = _get_bass(C)
    results = run_bass_kernel_spmd(nc, in_maps, list(range(_N_CORES))).results

    B = np.asarray(latents).shape[0]
    out = np.empty((B, _D), dtype=np.float32)
    for p in range(_P):
        yT = np.asarray(results[p]["outT"], dtype=np.float32).reshape(_D, C)
        out[order[offs[p] : offs[p + 1]]] = yT[:, : counts[p]].T
    return out
